# revision 53
# baseline (speedup 1.0000x reference)
"""nn_CausalSelfAttention_88854283420050 — Bass/Tile kernel for 8 trn2 cores.

Sharding: tensor-parallel over heads (H=16 -> 2 heads per core).
Each core computes, for its 2 heads: the qkv projection (columns of
c_attn), per-head LayerNorm + RoPE, causal attention, and a partial
output projection y_c = O_heads @ W_proj[:, head cols].T.  The host
sums the 8 partial projections (row-parallel c_proj) and adds b_proj.

v5 (232us baseline -> ~201us): one fused stream engineered so the PE
never idles and its HAM clock-gate stays at 2.4 GHz end-to-end.
  - attention blocks (h, 0..2) are hosted INSIDE the qkv phase at tiles
    4,6,8,10,12,14: each hosted block consumes its host tile's qkv
    matmuls as PE filler between exp-paced score steps.  Blocks (h,3)
    + the output projection run after, with projection thunks as the
    s-step filler and a 16-thunk prefill covering the entry wait.
  - startup: arrival-ordered coalesced DMA (w in 5 slices, rope/x1
    slotted between) + ramped dummy-matmul fill (2/4/6 per dt) matches
    PE progress to the HBM supply rate, so the HAM warms at ~10.5us
    and never re-throttles.
  - LN rstd via quake-seed rsqrt + 1 Newton step on DVE (no ACT Sqrt
    -> the single exp_and_others ACT table serves the whole kernel).
  - L (softmax denominator) matmul uses an all-ones [128,128]
    stationary so L arrives pre-broadcast: 1/L normalize is recip +
    one multiply on DVE, and block tails retire eagerly.
  - PSUM: psA2 psB1 psT1 psS2 psO1 psL1 = 8 banks in the fused phase
    (tiles 0-2 borrow idle psS slots for psB double-buffering); psY3
    opens after the qkv pools close.  Projection drains split
    ACT/DVE; the final block DMAs y per 512-col chunk as it drains.
  - fp16 operands end-to-end (fp32 PSUM accumulation); exp carries a
    constant -4 bias so fp16 attention weights cannot overflow (the
    1/L normalization cancels it); per-head channel sums ride as 4
    extra qkv columns so the LN mean is free; causal structure
    exploited at 128-col granularity.
"""
import math
import os
import sys

sys.path.insert(0, "/opt/trn_rl_repo")

import numpy as np
from concourse import bacc, mybir, tile
from concourse import bass_utils

T, D, H, C = 2048, 2048, 16, 128
EPS = 1e-6
NCORES = 8
HPC = H // NCORES  # heads per core
DT = 16            # contraction tiles (no bias row)
F16 = mybir.dt.float16
F32 = mybir.dt.float32
I32 = mybir.dt.int32
AF = mybir.ActivationFunctionType
ALU = mybir.AluOpType

NT = T // 128      # 16 row tiles
NB = T // 512      # 4 big t-blocks
WQ = 6 * C + 4     # qkv weight cols + 4 per-head sum cols
EXPB = -4.0        # constant exp bias; cancelled by 1/L
NWARM = 8          # HAM warm-up dummy matmuls before any DMA lands
RSQRT_MAGIC = 0x5f3759df + 1   # quake seed; +1 because we use ~x + (m+1)

# which attention block is hosted inside which qkv tile (spread out so
# consecutive tiles' LN chains don't pile up behind two hosted blocks)
HOST = {4: (0, 0), 6: (1, 0), 8: (0, 1), 10: (1, 1), 12: (0, 2), 14: (1, 2)}

_NC_CACHE = {}
LAST_RESULT = None


def _build_program(with_bias):
    nc = bacc.Bacc("TRN2", target_bir_lowering=False, debug=False,
                   enable_asserts=True, num_devices=NCORES)

    xts = nc.dram_tensor("xts", [NT, 128, DT, 128], F16, kind="ExternalInput").ap()
    wts = nc.dram_tensor("wts", [DT, 128, WQ], F16, kind="ExternalInput").ap()
    rope = nc.dram_tensor("rope", [NT, 128, 1024], F16, kind="ExternalInput").ap()
    masks = nc.dram_tensor("masks", [128, 128], F16, kind="ExternalInput").ap()
    wpd = nc.dram_tensor("wpd", [128, HPC, D], F16, kind="ExternalInput").ap()
    ident = nc.dram_tensor("ident", [128, 128], F16, kind="ExternalInput").ap()
    if with_bias:
        biasq = nc.dram_tensor("biasq", [1, WQ], F16, kind="ExternalInput").ap()
        ones1r = nc.dram_tensor("ones1r", [1, 128], F16, kind="ExternalInput").ap()
    y = nc.dram_tensor("y", [T, D], F16, kind="ExternalOutput").ap()

    sc = 1.0 / math.sqrt(C)

    with tile.TileContext(nc) as tc:
        with tc.tile_pool(name="res", bufs=1) as res:
            qT = res.tile([128, HPC, T], F16, tag="qT")        # [c, h, t]
            kT = res.tile([128, HPC, T], F16, tag="kT")
            vv = res.tile([128, HPC, NT, C], F16, tag="vv")    # [s, h, stile, c]
            ot = res.tile([128, HPC, T], F16, tag="ot")        # [c, h, t]
            w_sb = res.tile([128, DT, WQ], F16, tag="w_sb")
            masks_sb = res.tile([128, 128], F16, tag="masks")
            wp_sb = res.tile([128, HPC, D], F16, tag="wp")
            ones128 = res.tile([128, 128], F16, tag="ones128")
            id_sb = res.tile([128, 128], F16, tag="ident")
            zeros_c = res.tile([128, 1], F32, tag="zeros_c")
            negb_c = res.tile([128, 1], F32, tag="negb_c")
            if with_bias:
                bias_sb = res.tile([1, WQ], F16, tag="bias_sb")
                ones_1r = res.tile([1, 128], F16, tag="ones_1r")

            nc.gpsimd.memset(zeros_c[:], 0.0)
            nc.gpsimd.memset(negb_c[:], EXPB)
            nc.gpsimd.memset(ones128[:], 1.0)
            scr1 = res.tile([128, 1], F32, tag="scr1")
            # table warmer: touching Exp now loads the one ACT table (it
            # also contains Square/Copy) during the startup DMA wait.
            nc.scalar.activation(scr1[:], zeros_c[:], AF.Exp)

            with (
                tc.tile_pool(name="psS", bufs=2, space="PSUM") as psSp,
                tc.tile_pool(name="psO", bufs=1, space="PSUM") as psOp,
                tc.tile_pool(name="psL", bufs=1, space="PSUM") as psLp,
                tc.tile_pool(name="aT", bufs=28) as aTp,
                tc.tile_pool(name="bsm", bufs=2) as bsmp,
            ):
                # ---- HAM warm-up: dummy matmuls during the startup DMA wait
                warm = psSp.tile([128, 512], F32, tag="psS", name="warm")
                for _ in range(24):
                    nc.tensor.matmul(warm[:, 0:128], ones128[:], ones128[:],
                                     start=True, stop=True)

                state = {"old": None}
                cfill = []

                def finish_old(st):
                    # eager tail: L arrives broadcast across partitions, so
                    # the normalize is recip + one multiply, all on DVE.
                    recL = bsmp.tile([128, 512], F32, tag="recL")
                    nc.vector.reciprocal_approx_fast(recL[:], st["Lps"][:])
                    nc.vector.tensor_tensor(
                        ot[:, st["h"], st["tb"] * 512:(st["tb"] + 1) * 512],
                        st["Ops"][:], recL[:], op=ALU.mult)

                def emit_lo(st):
                    s = st["idx"]
                    S_old = len(st["a"])
                    a, lo = st["a"][s]
                    nc.tensor.matmul(
                        st["Lps"][:, lo:512], ones128[:], a[:, lo:512],
                        start=(s == 0), stop=(s == S_old - 1))
                    nc.tensor.matmul(
                        st["Ops"][:, lo:512], vv[:, st["h"], s, :],
                        a[:, lo:512],
                        start=(s == 0), stop=(s == S_old - 1))
                    st["idx"] += 1
                    if st["idx"] == S_old:
                        finish_old(st)

                def drain_old():
                    while state["old"] is not None and \
                            state["old"]["idx"] < len(state["old"]["a"]):
                        emit_lo(state["old"])

                def attn_block(h, tb, filler=None):
                    S = 4 * (tb + 1)
                    qTs = qT[:, h, tb * 512:(tb + 1) * 512]
                    a_list = []
                    for s in range(S):
                        lo = (s - 4 * tb) * 128 if s >= 4 * tb else 0
                        if state["old"] is not None and \
                                state["old"]["idx"] < len(state["old"]["a"]):
                            emit_lo(state["old"])
                        stp = psSp.tile([128, 512], F32, tag="psS")
                        nc.tensor.matmul(
                            stp[:, lo:512],
                            kT[:, h, s * 128:(s + 1) * 128], qTs[:, lo:512],
                            start=True, stop=True)
                        a = aTp.tile([128, 512], F16, tag="aT")
                        nc.scalar.activation(
                            a[:, lo:512], stp[:, lo:512], AF.Exp,
                            bias=negb_c[:], scale=sc)
                        if s >= 4 * tb:
                            # only the [128,128] triangle needs masking
                            nc.vector.tensor_tensor(
                                a[:, lo:lo + 128], a[:, lo:lo + 128],
                                masks_sb[:], op=ALU.mult)
                        a_list.append((a, lo))
                        # PE filler between score matmuls: hosted qkv work
                        # (fused phase) or projection thunks (late phase).
                        if filler is not None:
                            for _ in range(2):
                                if filler:
                                    filler.pop(0)()
                        elif s >= 2:
                            for _ in range(2):
                                if cfill:
                                    cfill.pop(0)()
                    drain_old()
                    state["old"] = dict(
                        h=h, tb=tb, a=a_list, idx=0,
                        Lps=psLp.tile([128, 512], F32, tag="psL", name="Lps"),
                        Ops=psOp.tile([128, 512], F32, tag="psO", name="Ops"))

                # =========== fused phase: qkv + LN + RoPE (+ hosted attn) ====
                with (
                    tc.tile_pool(name="xcol", bufs=3) as xcolp,
                    tc.tile_pool(name="ropep", bufs=3) as ropep,
                    tc.tile_pool(name="qn", bufs=2) as qnp,
                    tc.tile_pool(name="psA", bufs=2, space="PSUM") as psAp,
                    tc.tile_pool(name="psB", bufs=1, space="PSUM") as psBp,
                    tc.tile_pool(name="psT", bufs=1, space="PSUM") as psTp,
                    tc.tile_pool(name="lnst", bufs=2) as lnstp,
                    tc.tile_pool(name="sq", bufs=2) as sqp,
                    tc.tile_pool(name="rot", bufs=2) as rotp,
                ):
                    qn_prev = None
                    xcol_t = {}
                    rope_t = {}
                    first_psum = {}

                    def issue_tile_dmas(tt, split=False):
                        xc = xcolp.tile([128, DT, 128], F16, tag="xcol",
                                        name="xcol")
                        rc = ropep.tile([128, 1024], F16, tag="rope",
                                        name="rope")
                        if split:
                            nc.sync.dma_start(xc[:, 0:4, :], xts[tt, :, 0:4, :])
                            nc.sync.dma_start(xc[:, 4:DT, :],
                                              xts[tt, :, 4:DT, :])
                        else:
                            nc.sync.dma_start(xc[:], xts[tt])
                        nc.sync.dma_start(rc[:], rope[tt])
                        xcol_t[tt] = xc
                        rope_t[tt] = rc

                    def transpose_out(qn, dve_copies=False):
                        # PE transpose of the finished qn tile into qT/kT.
                        # The final tile's drains go to DVE: the late
                        # phase's first score waits on them, and ACT is
                        # still chewing tile 15's LN squares.
                        psT = psTp.tile([128, 4, 128], F16, tag="psT")
                        tt_, qn_t = qn
                        for i in range(4):
                            nc.tensor.transpose(
                                psT[:, i, :], qn_t[:, i * 128:(i + 1) * 128],
                                id_sb[:])
                        if dve_copies:
                            nc.vector.tensor_copy(
                                qT[:, 0:2, tt_ * 128:(tt_ + 1) * 128],
                                psT[:, 0:2, :])
                            nc.vector.tensor_copy(
                                kT[:, 0:2, tt_ * 128:(tt_ + 1) * 128],
                                psT[:, 2:4, :])
                        else:
                            nc.scalar.activation(
                                qT[:, 0:2, tt_ * 128:(tt_ + 1) * 128],
                                psT[:, 0:2, :], AF.Copy)
                            nc.scalar.activation(
                                kT[:, 0:2, tt_ * 128:(tt_ + 1) * 128],
                                psT[:, 2:4, :], AF.Copy)

                    for tt in range(NT):
                        if tt == 0:
                            # startup: every DMA descriptor costs ~0.6us
                            # serially on the sync queue, so coalesce the
                            # weight chunks into 3 descriptors sized so each
                            # lands just before the PE needs it.
                            # arrival-ordered: the w stream is tile 0's
                            # critical path, but rope0/rope1/x1 must land
                            # before tile 0/1's LN chains and tile 1's qkv,
                            # so they slot between w slices instead of after.
                            wtr = wts.rearrange("d p w -> p d w")
                            nc.sync.dma_start(w_sb[:, 0:1, :], wtr[:, 0:1, :])
                            xc = xcolp.tile([128, DT, 128], F16, tag="xcol",
                                            name="xcol")
                            rc0 = ropep.tile([128, 1024], F16, tag="rope",
                                             name="rope")
                            xc1 = xcolp.tile([128, DT, 128], F16, tag="xcol",
                                             name="xcol")
                            rc1 = ropep.tile([128, 1024], F16, tag="rope",
                                             name="rope")
                            nc.sync.dma_start(xc[:, 0:4, :], xts[0, :, 0:4, :])
                            nc.sync.dma_start(w_sb[:, 1:5, :], wtr[:, 1:5, :])
                            nc.sync.dma_start(xc[:, 4:DT, :],
                                              xts[0, :, 4:DT, :])
                            nc.sync.dma_start(w_sb[:, 5:8, :], wtr[:, 5:8, :])
                            nc.sync.dma_start(rc0[:], rope[0])
                            nc.sync.dma_start(w_sb[:, 8:11, :],
                                              wtr[:, 8:11, :])
                            nc.sync.dma_start(w_sb[:, 11:DT, :],
                                              wtr[:, 11:DT, :])
                            nc.sync.dma_start(rc1[:], rope[1])
                            nc.sync.dma_start(xc1[:], xts[1])
                            xcol_t[0] = xc
                            rope_t[0] = rc0
                            xcol_t[1] = xc1
                            rope_t[1] = rc1
                            nc.sync.dma_start(id_sb[:], ident[:])
                            nc.sync.dma_start(masks_sb[:], masks[:])
                            nc.sync.dma_start(wp_sb[:], wpd[:])
                            if with_bias:
                                nc.sync.dma_start(bias_sb[:], biasq[:])
                                nc.sync.dma_start(ones_1r[:], ones1r[:])
                            issue_tile_dmas(2)
                        elif tt + 2 < NT:
                            issue_tile_dmas(tt + 2)

                        xcol = xcol_t.pop(tt)
                        rc = rope_t.pop(tt)

                        psA = psAp.tile([128, 512], F32, tag="psA")
                        if tt < 3:
                            # psS slots are idle until the first hosted
                            # block (tile 4): borrow them so psB is
                            # effectively double-buffered through the
                            # startup crunch.
                            psB = psSp.tile([128, 512], F32, tag="psS",
                                            name="psBx")[:, 0:260]
                        else:
                            psB = psBp.tile([128, 260], F32, tag="psB")

                        thunks = []
                        for dt0 in range(0, DT, 2):
                            def qkv_pair(dt0=dt0, tt=tt):
                                for dt in range(dt0, dt0 + 2):
                                    # no-dep filler: keeps the PE busy
                                    # through startup DMA supply stalls
                                    # and sustains the HAM busy window.
                                    ndum = 2 if dt < 4 else 4 if dt < 8 else 6
                                    if tt == 0:
                                        for _ in range(ndum):
                                            nc.tensor.matmul(
                                                warm[:, 0:128], ones128[:],
                                                ones128[:], start=True,
                                                stop=True)
                                    nc.tensor.matmul(
                                        psA[:], xcol[:, dt, :],
                                        w_sb[:, dt, 0:512],
                                        start=(dt == 0),
                                        stop=(dt == DT - 1 and not with_bias))
                                    nc.tensor.matmul(
                                        psB[:], xcol[:, dt, :],
                                        w_sb[:, dt, 512:772],
                                        start=(dt == 0),
                                        stop=(dt == DT - 1 and not with_bias))
                            thunks.append(qkv_pair)
                        if with_bias:
                            def bias_mm():
                                nc.tensor.matmul(
                                    psA[:], ones_1r[:], bias_sb[:, 0:512],
                                    start=False, stop=True)
                                nc.tensor.matmul(
                                    psB[:], ones_1r[:], bias_sb[:, 512:772],
                                    start=False, stop=True)
                            thunks.append(bias_mm)

                        if tt in HOST:
                            # transpose first so the hosted block's q/k tiles
                            # are complete, then run the block with this
                            # tile's qkv matmuls as its PE filler.
                            if qn_prev is not None:
                                transpose_out(qn_prev)
                                qn_prev = None
                            h, b = HOST[tt]
                            thunks[0]()
                            thunks.pop(0)
                            attn_block(h, b, filler=thunks)
                            while thunks:
                                thunks.pop(0)()
                        else:
                            for th in thunks:
                                th()
                            if qn_prev is not None:
                                transpose_out(qn_prev)
                                qn_prev = None

                        if tt in HOST:
                            # ACT is exp-loaded on hosted tiles; drain v on
                            # DVE so the next tile's psB matmul isn't held
                            # up behind the exps (psB is single-buffered).
                            nc.vector.tensor_copy(
                                vv[:, 0:2, tt, :],
                                psB[:, 0:256].rearrange("p (h c) -> p h c",
                                                        h=2))
                        else:
                            nc.scalar.activation(
                                vv[:, 0:2, tt, :],
                                psB[:, 0:256].rearrange("p (h c) -> p h c",
                                                        h=2),
                                AF.Copy)

                        st = lnstp.tile([128, 16], F32, tag="lnst")
                        # st cols: 0:4 -mu, 4:8 sumsq, 8:12 rstd, 12:16 mu*rstd... actually -mu*rstd
                        nc.vector.tensor_scalar(
                            st[:, 0:4], psB[:, 256:260], -1.0 / C, None,
                            op0=ALU.mult)
                        for i in range(4):
                            sq = sqp.tile([128, 128], F32, tag="sq")
                            nc.scalar.activation(
                                sq[:], psA[:, i * 128:(i + 1) * 128], AF.Square,
                                bias=zeros_c[:], accum_out=st[:, 4 + i:5 + i])
                        var = lnstp.tile([128, 4], F32, tag="var")
                        mu2 = lnstp.tile([128, 4], F32, tag="mu2")
                        yv = lnstp.tile([128, 4], F32, tag="yv")
                        tv = lnstp.tile([128, 4], F32, tag="tv")
                        nc.vector.tensor_scalar(
                            var[:], st[:, 4:8], 1.0 / C, EPS,
                            op0=ALU.mult, op1=ALU.add)
                        nc.vector.tensor_tensor(
                            mu2[:], st[:, 0:4], st[:, 0:4], op=ALU.mult)
                        nc.vector.tensor_tensor(
                            var[:], var[:], mu2[:], op=ALU.subtract)
                        # rstd = rsqrt(var) via quake seed + 1 Newton step
                        # (0.18% max err, cancels in the 2e-2 budget), all
                        # on DVE (no ACT Sqrt -> single ACT table).
                        nc.vector.tensor_scalar(
                            yv[:].bitcast(I32), var[:].bitcast(I32), 1, -1,
                            op0=ALU.logical_shift_right, op1=ALU.bitwise_xor)
                        nc.vector.tensor_scalar(
                            yv[:].bitcast(I32), yv[:].bitcast(I32),
                            RSQRT_MAGIC, None, op0=ALU.add)
                        nc.vector.tensor_tensor(
                            tv[:], var[:], yv[:], op=ALU.mult)
                        nc.vector.scalar_tensor_tensor(
                            tv[:], tv[:], -0.5, yv[:],
                            op0=ALU.mult, op1=ALU.mult)
                        nc.vector.scalar_tensor_tensor(
                            st[:, 8:12], tv[:], 1.5, yv[:],
                            op0=ALU.add, op1=ALU.mult)
                        nc.vector.tensor_tensor(
                            st[:, 12:16], st[:, 0:4], st[:, 8:12],
                            op=ALU.mult)
                        qn = qnp.tile([128, 512], F16, tag="qn")
                        for i in range(4):
                            nc.vector.tensor_scalar(
                                qn[:, i * 128:(i + 1) * 128],
                                psA[:, i * 128:(i + 1) * 128],
                                st[:, 8 + i:9 + i], st[:, 12 + i:13 + i],
                                op0=ALU.mult, op1=ALU.add)
                        # RoPE: the rotate-every-two sign is folded into the
                        # host-built sin table, so the swap-multiplies fuse
                        # into two strided TTs.
                        rot = rotp.tile([128, 512], F16, tag="rot")
                        qn3 = qn[:].rearrange("p (a b) -> p a b", b=2)
                        rot3 = rot[:].rearrange("p (a b) -> p a b", b=2)
                        rcs3 = rc[:, 512:1024].rearrange(
                            "p (a b) -> p a b", b=2)
                        nc.vector.tensor_tensor(
                            rot3[:, :, 0], qn3[:, :, 1], rcs3[:, :, 0],
                            op=ALU.mult)
                        nc.vector.tensor_tensor(
                            rot3[:, :, 1], qn3[:, :, 0], rcs3[:, :, 1],
                            op=ALU.mult)
                        nc.vector.tensor_tensor(
                            qn[:], qn[:], rc[:, 0:512], op=ALU.mult)
                        nc.vector.tensor_tensor(qn[:], qn[:], rot[:],
                                                op=ALU.add)
                        qn_prev = (tt, qn)

                    transpose_out(qn_prev, dve_copies=True)

                # =========== late phase: blocks (h,3) + output projection ====
                with (
                    tc.tile_pool(name="psY", bufs=3, space="PSUM") as psYp,
                    tc.tile_pool(name="ysb", bufs=2) as ysbp,
                ):
                    def queue_proj(tb, act_drains=True, split_dma=False):
                        for ttt in range(4 * tb, 4 * tb + 4):
                            box = {}
                            for db in range(NB):
                                def thunk(ttt=ttt, db=db, box=box):
                                    if db == 0:
                                        box["ysb"] = ysbp.tile(
                                            [128, 4, 512], F16, tag="ysb",
                                            name="ysb")
                                    yps = psYp.tile([128, 512], F32,
                                                    tag="psY", name="yps")
                                    for h in range(HPC):
                                        nc.tensor.matmul(
                                            yps[:],
                                            ot[:, h,
                                               ttt * 128:(ttt + 1) * 128],
                                            wp_sb[:, h,
                                                  db * 512:(db + 1) * 512],
                                            start=(h == 0),
                                            stop=(h == HPC - 1))
                                    # ACT carries the late blocks' exps, so
                                    # it only gets a quarter of the drains
                                    # (none in the final tail).
                                    if act_drains and db % (4 if not split_dma else 2) == 0:
                                        nc.scalar.activation(
                                            box["ysb"][:, db, :], yps[:],
                                            AF.Copy)
                                    else:
                                        nc.vector.tensor_copy(
                                            box["ysb"][:, db, :], yps[:])
                                    if split_dma:
                                        nc.sync.dma_start(
                                            y[ttt * 128:(ttt + 1) * 128,
                                              db * 512:(db + 1) * 512],
                                            box["ysb"][:, db, :])
                                    elif db == NB - 1:
                                        nc.sync.dma_start(
                                            y[ttt * 128:(ttt + 1) * 128, :],
                                            box["ysb"][:].rearrange(
                                                "p a b -> p (a b)"))
                                cfill.append(thunk)

                    queue_proj(0)
                    queue_proj(1)
                    # front-load ready PE work before block (0,3)'s first
                    # score, which must wait for tile 15's LN + transpose
                    # copies — otherwise the in-order PE queue stalls ~5us
                    # behind it (and the HAM re-throttles).  Projection
                    # thunks go first (no recent deps); block (1,2)'s L/O
                    # drain follows so its DVE tail lands before block
                    # (0,3) allocates psO/psL (WAR).
                    for _ in range(16):
                        if cfill:
                            cfill.pop(0)()
                    drain_old()
                    attn_block(0, NB - 1)
                    queue_proj(2)
                    attn_block(1, NB - 1)
                    drain_old()
                    queue_proj(NB - 1, act_drains=True, split_dma=True)
                    while cfill:
                        cfill.pop(0)()

    nc.compile()
    return nc


def _host_prep(x, W_attn, b_attn, W_proj, q_ln_w, k_ln_w):
    f = np.float32
    h16 = np.float16

    # x pretiled: xts[tt, p, a, j] = x[tt*128+j, a*128+p]
    x4 = x.reshape(NT, 128, DT, 128)          # [tt, j, a, p]
    xts = np.ascontiguousarray(
        x4.transpose(0, 3, 2, 1).astype(h16))  # [tt, p, a, j]

    inv = (1.0 / (10000.0 ** (np.arange(0, C, 2, dtype=f) / C))).astype(f)
    freqs = np.arange(T, dtype=f)[:, None] * inv[None, :]
    sin = np.repeat(np.sin(freqs), 2, axis=1).astype(f)
    cos = np.repeat(np.cos(freqs), 2, axis=1).astype(f)
    part = np.arange(C) ^ 1
    cos_q = cos * q_ln_w[None, :]
    sin_q = sin * q_ln_w[None, part]
    cos_k = cos * k_ln_w[None, :]
    sin_k = sin * k_ln_w[None, part]
    ropecos = np.concatenate([cos_q, cos_q, cos_k, cos_k], axis=1)
    ropesin = np.concatenate([sin_q, sin_q, sin_k, sin_k], axis=1)
    # fold the rotate-every-two sign into the sin table: the kernel
    # computes rot[2i] = qn[2i+1]*sin[2i], rot[2i+1] = qn[2i]*sin[2i+1],
    # so sin[2i] must carry the minus.
    ropesin[:, 0::2] *= -1.0
    ropetab = np.ascontiguousarray(
        np.concatenate([ropecos, ropesin], axis=1)
        .reshape(NT, 128, 1024).astype(h16))

    ss = np.arange(128)[:, None]
    ttm = np.arange(128)[None, :]
    masks = np.ascontiguousarray((ss <= ttm).astype(h16))

    with_bias = bool(np.any(b_attn != 0.0))

    shared = dict(xts=xts, rope=ropetab, masks=masks,
                  ident=np.eye(128, dtype=h16))
    if with_bias:
        shared["ones1r"] = np.ones((1, 128), h16)

    in_maps = []
    for c in range(NCORES):
        h0, h1 = HPC * c, HPC * c + 1
        rows = np.concatenate([
            np.arange(h0 * C, (h0 + 1) * C),
            np.arange(h1 * C, (h1 + 1) * C),
            D + np.arange(h0 * C, (h0 + 1) * C),
            D + np.arange(h1 * C, (h1 + 1) * C),
            2 * D + np.arange(h0 * C, (h0 + 1) * C),
            2 * D + np.arange(h1 * C, (h1 + 1) * C),
        ])
        wq = W_attn[rows].T                    # [D, 768]
        # 4 extra columns: per-head channel sums of the q/k blocks so the
        # LN mean comes out of the qkv matmul directly.
        wsum = wq[:, 0:512].reshape(D, 4, 128).sum(axis=2)   # [D, 4]
        wqa = np.concatenate([wq, wsum], axis=1)             # [D, 772]
        wts = np.ascontiguousarray(
            wqa.reshape(DT, 128, WQ).astype(h16))
        wpc = np.stack(
            [W_proj[:, h0 * C:(h0 + 1) * C].T,
             W_proj[:, h1 * C:(h1 + 1) * C].T], axis=0)  # [2, 128, D]
        wpd = np.ascontiguousarray(wpc.transpose(1, 0, 2).astype(h16))
        m = dict(shared)
        m["wts"] = wts
        m["wpd"] = wpd
        if with_bias:
            ba = b_attn[rows]
            bs = ba[0:512].reshape(4, 128).sum(axis=1)
            m["biasq"] = np.ascontiguousarray(
                np.concatenate([ba, bs])[None, :]).astype(h16)
        in_maps.append(m)
    return in_maps, with_bias


def kernel(x, W_attn, b_attn, W_proj, b_proj, q_ln_w, k_ln_w):
    global LAST_RESULT
    f = np.float32
    x = np.ascontiguousarray(np.asarray(x, f))
    W_attn = np.ascontiguousarray(np.asarray(W_attn, f))
    b_attn = np.ascontiguousarray(np.asarray(b_attn, f))
    W_proj = np.ascontiguousarray(np.asarray(W_proj, f))
    b_proj = np.ascontiguousarray(np.asarray(b_proj, f))
    q_ln_w = np.ascontiguousarray(np.asarray(q_ln_w, f))
    k_ln_w = np.ascontiguousarray(np.asarray(k_ln_w, f))

    in_maps, with_bias = _host_prep(x, W_attn, b_attn, W_proj, q_ln_w, k_ln_w)
    if with_bias not in _NC_CACHE:
        _NC_CACHE[with_bias] = _build_program(with_bias)
    nc = _NC_CACHE[with_bias]

    res = bass_utils.run_bass_kernel_spmd(
        nc, in_maps, core_ids=list(range(NCORES)),
        trace=bool(os.environ.get("BASS_TRACE")))
    LAST_RESULT = res

    y = np.zeros((T, D), np.float32)
    for rmap in res.results:
        y += rmap["y"].astype(np.float32)
    y += b_proj[None, :]
    return y


# revision 54
# speedup vs baseline: 1.0304x; 1.0304x over previous
"""nn_CausalSelfAttention_88854283420050 — Bass/Tile kernel for 8 trn2 cores.

Sharding: tensor-parallel over heads (H=16 -> 2 heads per core).
Each core computes, for its 2 heads: the qkv projection (columns of
c_attn), per-head LayerNorm + RoPE, causal attention, and a partial
output projection y_c = O_heads @ W_proj[:, head cols].T.  The host
sums the 8 partial projections (row-parallel c_proj) and adds b_proj.

v5 (232us baseline -> ~201us): one fused stream engineered so the PE
never idles and its HAM clock-gate stays at 2.4 GHz end-to-end.
  - attention blocks (h, 0..2) are hosted INSIDE the qkv phase at tiles
    4,6,8,10,12,14: each hosted block consumes its host tile's qkv
    matmuls as PE filler between exp-paced score steps.  Blocks (h,3)
    + the output projection run after, with projection thunks as the
    s-step filler and a 16-thunk prefill covering the entry wait.
  - startup: arrival-ordered coalesced DMA (w in 5 slices, rope/x1
    slotted between) + ramped dummy-matmul fill (2/4/6 per dt) matches
    PE progress to the HBM supply rate, so the HAM warms at ~10.5us
    and never re-throttles.
  - LN rstd via quake-seed rsqrt + 1 Newton step on DVE (no ACT Sqrt
    -> the single exp_and_others ACT table serves the whole kernel).
  - L (softmax denominator) matmul uses an all-ones [128,128]
    stationary so L arrives pre-broadcast: 1/L normalize is recip +
    one multiply on DVE, and block tails retire eagerly.
  - PSUM: psA2 psB1 psT1 psS2 psO1 psL1 = 8 banks in the fused phase
    (tiles 0-2 borrow idle psS slots for psB double-buffering); psY3
    opens after the qkv pools close.  Projection drains split
    ACT/DVE; the final block DMAs y per 512-col chunk as it drains.
  - fp16 operands end-to-end (fp32 PSUM accumulation); exp carries a
    constant -4 bias so fp16 attention weights cannot overflow (the
    1/L normalization cancels it); per-head channel sums ride as 4
    extra qkv columns so the LN mean is free; causal structure
    exploited at 128-col granularity.
"""
import math
import os
import sys

sys.path.insert(0, "/opt/trn_rl_repo")

import numpy as np
from concourse import bacc, mybir, tile
from concourse import bass_utils

T, D, H, C = 2048, 2048, 16, 128
EPS = 1e-6
NCORES = 8
HPC = H // NCORES  # heads per core
DT = 16            # contraction tiles (no bias row)
F16 = mybir.dt.float16
F32 = mybir.dt.float32
I32 = mybir.dt.int32
AF = mybir.ActivationFunctionType
ALU = mybir.AluOpType

NT = T // 128      # 16 row tiles
NB = T // 512      # 4 big t-blocks
WQ = 6 * C + 4     # qkv weight cols + 4 per-head sum cols
EXPB = -4.0        # constant exp bias; cancelled by 1/L
NWARM = 8          # HAM warm-up dummy matmuls before any DMA lands
RSQRT_MAGIC = 0x5f3759df + 1   # quake seed; +1 because we use ~x + (m+1)

# which attention block is hosted inside which qkv tile (spread out so
# consecutive tiles' LN chains don't pile up behind two hosted blocks)
HOST = {4: (0, 0), 6: (1, 0), 8: (0, 1), 10: (1, 1), 12: (0, 2), 14: (1, 2)}

_NC_CACHE = {}
LAST_RESULT = None


def _build_program(with_bias):
    nc = bacc.Bacc("TRN2", target_bir_lowering=False, debug=False,
                   enable_asserts=True, num_devices=NCORES)

    xts = nc.dram_tensor("xts", [NT, 128, DT, 128], F16, kind="ExternalInput").ap()
    wts = nc.dram_tensor("wts", [DT, 128, WQ], F16, kind="ExternalInput").ap()
    rope = nc.dram_tensor("rope", [NT, 128, 1024], F16, kind="ExternalInput").ap()
    masks = nc.dram_tensor("masks", [128, 128], F16, kind="ExternalInput").ap()
    wpd = nc.dram_tensor("wpd", [128, HPC, D], F16, kind="ExternalInput").ap()
    ident = nc.dram_tensor("ident", [128, 128], F16, kind="ExternalInput").ap()
    if with_bias:
        biasq = nc.dram_tensor("biasq", [1, WQ], F16, kind="ExternalInput").ap()
        ones1r = nc.dram_tensor("ones1r", [1, 128], F16, kind="ExternalInput").ap()
    y = nc.dram_tensor("y", [T, D], F16, kind="ExternalOutput").ap()

    sc = 1.0 / math.sqrt(C)

    with tile.TileContext(nc) as tc:
        with tc.tile_pool(name="res", bufs=1) as res:
            qT = res.tile([128, HPC, T], F16, tag="qT")        # [c, h, t]
            kT = res.tile([128, HPC, T], F16, tag="kT")
            vv = res.tile([128, HPC, NT, C], F16, tag="vv")    # [s, h, stile, c]
            ot = res.tile([128, HPC, T], F16, tag="ot")        # [c, h, t]
            w_sb = res.tile([128, DT, WQ], F16, tag="w_sb")
            masks_sb = res.tile([128, 128], F16, tag="masks")
            wp_sb = res.tile([128, HPC, D], F16, tag="wp")
            ones128 = res.tile([128, 128], F16, tag="ones128")
            id_sb = res.tile([128, 128], F16, tag="ident")
            zeros_c = res.tile([128, 1], F32, tag="zeros_c")
            negb_c = res.tile([128, 1], F32, tag="negb_c")
            if with_bias:
                bias_sb = res.tile([1, WQ], F16, tag="bias_sb")
                ones_1r = res.tile([1, 128], F16, tag="ones_1r")

            nc.gpsimd.memset(zeros_c[:], 0.0)
            nc.gpsimd.memset(negb_c[:], EXPB)
            nc.gpsimd.memset(ones128[:], 1.0)
            scr1 = res.tile([128, 1], F32, tag="scr1")
            # table warmer: touching Exp now loads the one ACT table (it
            # also contains Square/Copy) during the startup DMA wait.
            nc.scalar.activation(scr1[:], zeros_c[:], AF.Exp)

            with (
                tc.tile_pool(name="psS", bufs=2, space="PSUM") as psSp,
                tc.tile_pool(name="psO", bufs=1, space="PSUM") as psOp,
                tc.tile_pool(name="psL", bufs=1, space="PSUM") as psLp,
                tc.tile_pool(name="aT", bufs=28) as aTp,
                tc.tile_pool(name="bsm", bufs=2) as bsmp,
            ):
                # ---- HAM warm-up: dummy matmuls during the startup DMA wait
                warm = psSp.tile([128, 512], F32, tag="psS", name="warm")
                for _ in range(24):
                    nc.tensor.matmul(warm[:, 0:128], ones128[:], ones128[:],
                                     start=True, stop=True)

                state = {"old": None}
                cfill = []

                def finish_old(st):
                    # eager tail: L arrives broadcast across partitions, so
                    # the normalize is recip + one multiply, all on DVE.
                    recL = bsmp.tile([128, 512], F32, tag="recL")
                    nc.vector.reciprocal_approx_fast(recL[:], st["Lps"][:])
                    nc.vector.tensor_tensor(
                        ot[:, st["h"], st["tb"] * 512:(st["tb"] + 1) * 512],
                        st["Ops"][:], recL[:], op=ALU.mult)

                def emit_lo(st):
                    s = st["idx"]
                    S_old = len(st["a"])
                    a, lo = st["a"][s]
                    nc.tensor.matmul(
                        st["Lps"][:, lo:512], ones128[:], a[:, lo:512],
                        start=(s == 0), stop=(s == S_old - 1))
                    nc.tensor.matmul(
                        st["Ops"][:, lo:512], vv[:, st["h"], s, :],
                        a[:, lo:512],
                        start=(s == 0), stop=(s == S_old - 1))
                    st["idx"] += 1
                    if st["idx"] == S_old:
                        finish_old(st)

                def drain_old():
                    while state["old"] is not None and \
                            state["old"]["idx"] < len(state["old"]["a"]):
                        emit_lo(state["old"])

                def attn_block(h, tb, filler=None):
                    S = 4 * (tb + 1)
                    qTs = qT[:, h, tb * 512:(tb + 1) * 512]
                    a_list = []
                    for s in range(S):
                        lo = (s - 4 * tb) * 128 if s >= 4 * tb else 0
                        if state["old"] is not None and \
                                state["old"]["idx"] < len(state["old"]["a"]):
                            emit_lo(state["old"])
                        stp = psSp.tile([128, 512], F32, tag="psS")
                        nc.tensor.matmul(
                            stp[:, lo:512],
                            kT[:, h, s * 128:(s + 1) * 128], qTs[:, lo:512],
                            start=True, stop=True)
                        a = aTp.tile([128, 512], F16, tag="aT")
                        nc.scalar.activation(
                            a[:, lo:512], stp[:, lo:512], AF.Exp,
                            bias=negb_c[:], scale=sc)
                        if s >= 4 * tb:
                            # only the [128,128] triangle needs masking
                            nc.vector.tensor_tensor(
                                a[:, lo:lo + 128], a[:, lo:lo + 128],
                                masks_sb[:], op=ALU.mult)
                        a_list.append((a, lo))
                        # PE filler between score matmuls: hosted qkv work
                        # (fused phase) or projection thunks (late phase).
                        if filler is not None:
                            for _ in range(2):
                                if filler:
                                    filler.pop(0)()
                        elif s >= 2:
                            for _ in range(2):
                                if cfill:
                                    cfill.pop(0)()
                    drain_old()
                    state["old"] = dict(
                        h=h, tb=tb, a=a_list, idx=0,
                        Lps=psLp.tile([128, 512], F32, tag="psL", name="Lps"),
                        Ops=psOp.tile([128, 512], F32, tag="psO", name="Ops"))

                # =========== fused phase: qkv + LN + RoPE (+ hosted attn) ====
                with (
                    tc.tile_pool(name="xcol", bufs=3) as xcolp,
                    tc.tile_pool(name="ropep", bufs=3) as ropep,
                    tc.tile_pool(name="qn", bufs=2) as qnp,
                    tc.tile_pool(name="psA", bufs=2, space="PSUM") as psAp,
                    tc.tile_pool(name="psB", bufs=1, space="PSUM") as psBp,
                    tc.tile_pool(name="psT", bufs=1, space="PSUM") as psTp,
                    tc.tile_pool(name="lnst", bufs=2) as lnstp,
                    tc.tile_pool(name="sq", bufs=2) as sqp,
                    tc.tile_pool(name="rot", bufs=2) as rotp,
                ):
                    qn_prev = None
                    xcol_t = {}
                    rope_t = {}
                    first_psum = {}

                    def issue_tile_dmas(tt, split=False):
                        xc = xcolp.tile([128, DT, 128], F16, tag="xcol",
                                        name="xcol")
                        rc = ropep.tile([128, 1024], F16, tag="rope",
                                        name="rope")
                        if split:
                            nc.sync.dma_start(xc[:, 0:4, :], xts[tt, :, 0:4, :])
                            nc.sync.dma_start(xc[:, 4:DT, :],
                                              xts[tt, :, 4:DT, :])
                        else:
                            nc.sync.dma_start(xc[:], xts[tt])
                        nc.sync.dma_start(rc[:], rope[tt])
                        xcol_t[tt] = xc
                        rope_t[tt] = rc

                    def transpose_out(qn, dve_copies=False):
                        # PE transpose of the finished qn tile into qT/kT.
                        # The final tile's drains go to DVE: the late
                        # phase's first score waits on them, and ACT is
                        # still chewing tile 15's LN squares.
                        psT = psTp.tile([128, 4, 128], F16, tag="psT")
                        tt_, qn_t = qn
                        for i in range(4):
                            nc.tensor.transpose(
                                psT[:, i, :], qn_t[:, i * 128:(i + 1) * 128],
                                id_sb[:])
                        if dve_copies:
                            nc.vector.tensor_copy(
                                qT[:, 0:2, tt_ * 128:(tt_ + 1) * 128],
                                psT[:, 0:2, :])
                            nc.vector.tensor_copy(
                                kT[:, 0:2, tt_ * 128:(tt_ + 1) * 128],
                                psT[:, 2:4, :])
                        else:
                            nc.scalar.activation(
                                qT[:, 0:2, tt_ * 128:(tt_ + 1) * 128],
                                psT[:, 0:2, :], AF.Copy)
                            nc.scalar.activation(
                                kT[:, 0:2, tt_ * 128:(tt_ + 1) * 128],
                                psT[:, 2:4, :], AF.Copy)

                    for tt in range(NT):
                        if tt == 0:
                            # startup: every DMA descriptor costs ~0.6us
                            # serially on the sync queue, so coalesce the
                            # weight chunks into 3 descriptors sized so each
                            # lands just before the PE needs it.
                            # arrival-ordered: the w stream is tile 0's
                            # critical path, but rope0/rope1/x1 must land
                            # before tile 0/1's LN chains and tile 1's qkv,
                            # so they slot between w slices instead of after.
                            wtr = wts.rearrange("d p w -> p d w")
                            nc.sync.dma_start(w_sb[:, 0:1, :], wtr[:, 0:1, :])
                            xc = xcolp.tile([128, DT, 128], F16, tag="xcol",
                                            name="xcol")
                            rc0 = ropep.tile([128, 1024], F16, tag="rope",
                                             name="rope")
                            xc1 = xcolp.tile([128, DT, 128], F16, tag="xcol",
                                             name="xcol")
                            rc1 = ropep.tile([128, 1024], F16, tag="rope",
                                             name="rope")
                            nc.sync.dma_start(xc[:, 0:4, :], xts[0, :, 0:4, :])
                            nc.sync.dma_start(w_sb[:, 1:5, :], wtr[:, 1:5, :])
                            nc.sync.dma_start(xc[:, 4:DT, :],
                                              xts[0, :, 4:DT, :])
                            nc.sync.dma_start(w_sb[:, 5:8, :], wtr[:, 5:8, :])
                            nc.sync.dma_start(rc0[:], rope[0])
                            nc.sync.dma_start(w_sb[:, 8:11, :],
                                              wtr[:, 8:11, :])
                            nc.sync.dma_start(w_sb[:, 11:DT, :],
                                              wtr[:, 11:DT, :])
                            nc.sync.dma_start(rc1[:], rope[1])
                            nc.sync.dma_start(xc1[:], xts[1])
                            xcol_t[0] = xc
                            rope_t[0] = rc0
                            xcol_t[1] = xc1
                            rope_t[1] = rc1
                            nc.sync.dma_start(id_sb[:], ident[:])
                            nc.sync.dma_start(masks_sb[:], masks[:])
                            nc.sync.dma_start(wp_sb[:], wpd[:])
                            if with_bias:
                                nc.sync.dma_start(bias_sb[:], biasq[:])
                                nc.sync.dma_start(ones_1r[:], ones1r[:])
                            issue_tile_dmas(2)
                        elif tt + 2 < NT:
                            issue_tile_dmas(tt + 2)

                        xcol = xcol_t.pop(tt)
                        rc = rope_t.pop(tt)

                        psA = psAp.tile([128, 512], F32, tag="psA")
                        if tt < 3:
                            # psS slots are idle until the first hosted
                            # block (tile 4): borrow them so psB is
                            # effectively double-buffered through the
                            # startup crunch.
                            psB = psSp.tile([128, 512], F32, tag="psS",
                                            name="psBx")[:, 0:260]
                        else:
                            psB = psBp.tile([128, 260], F32, tag="psB")

                        thunks = []
                        for dt0 in range(0, DT, 2):
                            def qkv_pair(dt0=dt0, tt=tt):
                                for dt in range(dt0, dt0 + 2):
                                    # no-dep filler: keeps the PE busy
                                    # through startup DMA supply stalls
                                    # and sustains the HAM busy window.
                                    ndum = 2 if dt < 4 else 5 if dt < 8 else 8
                                    if tt == 0:
                                        for _ in range(ndum):
                                            nc.tensor.matmul(
                                                warm[:, 0:128], ones128[:],
                                                ones128[:], start=True,
                                                stop=True)
                                    nc.tensor.matmul(
                                        psA[:], xcol[:, dt, :],
                                        w_sb[:, dt, 0:512],
                                        start=(dt == 0),
                                        stop=(dt == DT - 1 and not with_bias))
                                    nc.tensor.matmul(
                                        psB[:], xcol[:, dt, :],
                                        w_sb[:, dt, 512:772],
                                        start=(dt == 0),
                                        stop=(dt == DT - 1 and not with_bias))
                            thunks.append(qkv_pair)
                        if with_bias:
                            def bias_mm():
                                nc.tensor.matmul(
                                    psA[:], ones_1r[:], bias_sb[:, 0:512],
                                    start=False, stop=True)
                                nc.tensor.matmul(
                                    psB[:], ones_1r[:], bias_sb[:, 512:772],
                                    start=False, stop=True)
                            thunks.append(bias_mm)

                        if tt in HOST:
                            # transpose first so the hosted block's q/k tiles
                            # are complete, then run the block with this
                            # tile's qkv matmuls as its PE filler.
                            if qn_prev is not None:
                                transpose_out(qn_prev)
                                qn_prev = None
                            h, b = HOST[tt]
                            thunks[0]()
                            thunks.pop(0)
                            attn_block(h, b, filler=thunks)
                            while thunks:
                                thunks.pop(0)()
                        else:
                            for th in thunks:
                                th()
                            if qn_prev is not None:
                                transpose_out(qn_prev)
                                qn_prev = None

                        if tt in HOST:
                            # ACT is exp-loaded on hosted tiles; drain v on
                            # DVE so the next tile's psB matmul isn't held
                            # up behind the exps (psB is single-buffered).
                            nc.vector.tensor_copy(
                                vv[:, 0:2, tt, :],
                                psB[:, 0:256].rearrange("p (h c) -> p h c",
                                                        h=2))
                        else:
                            nc.scalar.activation(
                                vv[:, 0:2, tt, :],
                                psB[:, 0:256].rearrange("p (h c) -> p h c",
                                                        h=2),
                                AF.Copy)

                        st = lnstp.tile([128, 16], F32, tag="lnst")
                        # st cols: 0:4 -mu, 4:8 sumsq, 8:12 rstd, 12:16 mu*rstd... actually -mu*rstd
                        nc.vector.tensor_scalar(
                            st[:, 0:4], psB[:, 256:260], -1.0 / C, None,
                            op0=ALU.mult)
                        for i in range(4):
                            sq = sqp.tile([128, 128], F32, tag="sq")
                            nc.scalar.activation(
                                sq[:], psA[:, i * 128:(i + 1) * 128], AF.Square,
                                bias=zeros_c[:], accum_out=st[:, 4 + i:5 + i])
                        var = lnstp.tile([128, 4], F32, tag="var")
                        mu2 = lnstp.tile([128, 4], F32, tag="mu2")
                        yv = lnstp.tile([128, 4], F32, tag="yv")
                        tv = lnstp.tile([128, 4], F32, tag="tv")
                        nc.vector.tensor_scalar(
                            var[:], st[:, 4:8], 1.0 / C, EPS,
                            op0=ALU.mult, op1=ALU.add)
                        nc.vector.tensor_tensor(
                            mu2[:], st[:, 0:4], st[:, 0:4], op=ALU.mult)
                        nc.vector.tensor_tensor(
                            var[:], var[:], mu2[:], op=ALU.subtract)
                        # rstd = rsqrt(var) via quake seed + 1 Newton step
                        # (0.18% max err, cancels in the 2e-2 budget), all
                        # on DVE (no ACT Sqrt -> single ACT table).
                        nc.vector.tensor_scalar(
                            yv[:].bitcast(I32), var[:].bitcast(I32), 1, -1,
                            op0=ALU.logical_shift_right, op1=ALU.bitwise_xor)
                        nc.vector.tensor_scalar(
                            yv[:].bitcast(I32), yv[:].bitcast(I32),
                            RSQRT_MAGIC, None, op0=ALU.add)
                        nc.vector.tensor_tensor(
                            tv[:], var[:], yv[:], op=ALU.mult)
                        nc.vector.scalar_tensor_tensor(
                            tv[:], tv[:], -0.5, yv[:],
                            op0=ALU.mult, op1=ALU.mult)
                        nc.vector.scalar_tensor_tensor(
                            st[:, 8:12], tv[:], 1.5, yv[:],
                            op0=ALU.add, op1=ALU.mult)
                        nc.vector.tensor_tensor(
                            st[:, 12:16], st[:, 0:4], st[:, 8:12],
                            op=ALU.mult)
                        qn = qnp.tile([128, 512], F16, tag="qn")
                        for i in range(4):
                            nc.vector.tensor_scalar(
                                qn[:, i * 128:(i + 1) * 128],
                                psA[:, i * 128:(i + 1) * 128],
                                st[:, 8 + i:9 + i], st[:, 12 + i:13 + i],
                                op0=ALU.mult, op1=ALU.add)
                        # RoPE: the rotate-every-two sign is folded into the
                        # host-built sin table, so the swap-multiplies fuse
                        # into two strided TTs.
                        rot = rotp.tile([128, 512], F16, tag="rot")
                        qn3 = qn[:].rearrange("p (a b) -> p a b", b=2)
                        rot3 = rot[:].rearrange("p (a b) -> p a b", b=2)
                        rcs3 = rc[:, 512:1024].rearrange(
                            "p (a b) -> p a b", b=2)
                        nc.vector.tensor_tensor(
                            rot3[:, :, 0], qn3[:, :, 1], rcs3[:, :, 0],
                            op=ALU.mult)
                        nc.vector.tensor_tensor(
                            rot3[:, :, 1], qn3[:, :, 0], rcs3[:, :, 1],
                            op=ALU.mult)
                        nc.vector.tensor_tensor(
                            qn[:], qn[:], rc[:, 0:512], op=ALU.mult)
                        nc.vector.tensor_tensor(qn[:], qn[:], rot[:],
                                                op=ALU.add)
                        qn_prev = (tt, qn)

                    transpose_out(qn_prev, dve_copies=True)

                # =========== late phase: blocks (h,3) + output projection ====
                with (
                    tc.tile_pool(name="psY", bufs=3, space="PSUM") as psYp,
                    tc.tile_pool(name="ysb", bufs=2) as ysbp,
                ):
                    def queue_proj(tb, act_drains=True, split_dma=False):
                        for ttt in range(4 * tb, 4 * tb + 4):
                            box = {}
                            for db in range(NB):
                                def thunk(ttt=ttt, db=db, box=box):
                                    if db == 0:
                                        box["ysb"] = ysbp.tile(
                                            [128, 4, 512], F16, tag="ysb",
                                            name="ysb")
                                    yps = psYp.tile([128, 512], F32,
                                                    tag="psY", name="yps")
                                    for h in range(HPC):
                                        nc.tensor.matmul(
                                            yps[:],
                                            ot[:, h,
                                               ttt * 128:(ttt + 1) * 128],
                                            wp_sb[:, h,
                                                  db * 512:(db + 1) * 512],
                                            start=(h == 0),
                                            stop=(h == HPC - 1))
                                    # ACT carries the late blocks' exps, so
                                    # it only gets a quarter of the drains
                                    # (none in the final tail).
                                    if act_drains and db % (4 if not split_dma else 2) == 0:
                                        nc.scalar.activation(
                                            box["ysb"][:, db, :], yps[:],
                                            AF.Copy)
                                    else:
                                        nc.vector.tensor_copy(
                                            box["ysb"][:, db, :], yps[:])
                                    if split_dma:
                                        nc.sync.dma_start(
                                            y[ttt * 128:(ttt + 1) * 128,
                                              db * 512:(db + 1) * 512],
                                            box["ysb"][:, db, :])
                                    elif db == NB - 1:
                                        nc.sync.dma_start(
                                            y[ttt * 128:(ttt + 1) * 128, :],
                                            box["ysb"][:].rearrange(
                                                "p a b -> p (a b)"))
                                cfill.append(thunk)

                    queue_proj(0)
                    queue_proj(1)
                    # front-load ready PE work before block (0,3)'s first
                    # score, which must wait for tile 15's LN + transpose
                    # copies — otherwise the in-order PE queue stalls ~5us
                    # behind it (and the HAM re-throttles).  Projection
                    # thunks go first (no recent deps); block (1,2)'s L/O
                    # drain follows so its DVE tail lands before block
                    # (0,3) allocates psO/psL (WAR).
                    for _ in range(16):
                        if cfill:
                            cfill.pop(0)()
                    drain_old()
                    attn_block(0, NB - 1)
                    queue_proj(2)
                    attn_block(1, NB - 1)
                    drain_old()
                    queue_proj(NB - 1, act_drains=True, split_dma=True)
                    while cfill:
                        cfill.pop(0)()

    nc.compile()
    return nc


def _host_prep(x, W_attn, b_attn, W_proj, q_ln_w, k_ln_w):
    f = np.float32
    h16 = np.float16

    # x pretiled: xts[tt, p, a, j] = x[tt*128+j, a*128+p]
    x4 = x.reshape(NT, 128, DT, 128)          # [tt, j, a, p]
    xts = np.ascontiguousarray(
        x4.transpose(0, 3, 2, 1).astype(h16))  # [tt, p, a, j]

    inv = (1.0 / (10000.0 ** (np.arange(0, C, 2, dtype=f) / C))).astype(f)
    freqs = np.arange(T, dtype=f)[:, None] * inv[None, :]
    sin = np.repeat(np.sin(freqs), 2, axis=1).astype(f)
    cos = np.repeat(np.cos(freqs), 2, axis=1).astype(f)
    part = np.arange(C) ^ 1
    cos_q = cos * q_ln_w[None, :]
    sin_q = sin * q_ln_w[None, part]
    cos_k = cos * k_ln_w[None, :]
    sin_k = sin * k_ln_w[None, part]
    ropecos = np.concatenate([cos_q, cos_q, cos_k, cos_k], axis=1)
    ropesin = np.concatenate([sin_q, sin_q, sin_k, sin_k], axis=1)
    # fold the rotate-every-two sign into the sin table: the kernel
    # computes rot[2i] = qn[2i+1]*sin[2i], rot[2i+1] = qn[2i]*sin[2i+1],
    # so sin[2i] must carry the minus.
    ropesin[:, 0::2] *= -1.0
    ropetab = np.ascontiguousarray(
        np.concatenate([ropecos, ropesin], axis=1)
        .reshape(NT, 128, 1024).astype(h16))

    ss = np.arange(128)[:, None]
    ttm = np.arange(128)[None, :]
    masks = np.ascontiguousarray((ss <= ttm).astype(h16))

    with_bias = bool(np.any(b_attn != 0.0))

    shared = dict(xts=xts, rope=ropetab, masks=masks,
                  ident=np.eye(128, dtype=h16))
    if with_bias:
        shared["ones1r"] = np.ones((1, 128), h16)

    in_maps = []
    for c in range(NCORES):
        h0, h1 = HPC * c, HPC * c + 1
        rows = np.concatenate([
            np.arange(h0 * C, (h0 + 1) * C),
            np.arange(h1 * C, (h1 + 1) * C),
            D + np.arange(h0 * C, (h0 + 1) * C),
            D + np.arange(h1 * C, (h1 + 1) * C),
            2 * D + np.arange(h0 * C, (h0 + 1) * C),
            2 * D + np.arange(h1 * C, (h1 + 1) * C),
        ])
        wq = W_attn[rows].T                    # [D, 768]
        # 4 extra columns: per-head channel sums of the q/k blocks so the
        # LN mean comes out of the qkv matmul directly.
        wsum = wq[:, 0:512].reshape(D, 4, 128).sum(axis=2)   # [D, 4]
        wqa = np.concatenate([wq, wsum], axis=1)             # [D, 772]
        wts = np.ascontiguousarray(
            wqa.reshape(DT, 128, WQ).astype(h16))
        wpc = np.stack(
            [W_proj[:, h0 * C:(h0 + 1) * C].T,
             W_proj[:, h1 * C:(h1 + 1) * C].T], axis=0)  # [2, 128, D]
        wpd = np.ascontiguousarray(wpc.transpose(1, 0, 2).astype(h16))
        m = dict(shared)
        m["wts"] = wts
        m["wpd"] = wpd
        if with_bias:
            ba = b_attn[rows]
            bs = ba[0:512].reshape(4, 128).sum(axis=1)
            m["biasq"] = np.ascontiguousarray(
                np.concatenate([ba, bs])[None, :]).astype(h16)
        in_maps.append(m)
    return in_maps, with_bias


def kernel(x, W_attn, b_attn, W_proj, b_proj, q_ln_w, k_ln_w):
    global LAST_RESULT
    f = np.float32
    x = np.ascontiguousarray(np.asarray(x, f))
    W_attn = np.ascontiguousarray(np.asarray(W_attn, f))
    b_attn = np.ascontiguousarray(np.asarray(b_attn, f))
    W_proj = np.ascontiguousarray(np.asarray(W_proj, f))
    b_proj = np.ascontiguousarray(np.asarray(b_proj, f))
    q_ln_w = np.ascontiguousarray(np.asarray(q_ln_w, f))
    k_ln_w = np.ascontiguousarray(np.asarray(k_ln_w, f))

    in_maps, with_bias = _host_prep(x, W_attn, b_attn, W_proj, q_ln_w, k_ln_w)
    if with_bias not in _NC_CACHE:
        _NC_CACHE[with_bias] = _build_program(with_bias)
    nc = _NC_CACHE[with_bias]

    res = bass_utils.run_bass_kernel_spmd(
        nc, in_maps, core_ids=list(range(NCORES)),
        trace=bool(os.environ.get("BASS_TRACE")))
    LAST_RESULT = res

    y = np.zeros((T, D), np.float32)
    for rmap in res.results:
        y += rmap["y"].astype(np.float32)
    y += b_proj[None, :]
    return y
